# revision 11
# baseline (speedup 1.0000x reference)
"""Block-diagonal (per-graph) multi-head attention for Trainium2, SPMD over 8 cores.

Problem: nn_MultiHeadAttention (sparse_attention). N=6144 nodes in B=16 graphs
of 384 nodes each (batch ids are sorted and uniform), d_model=256, 8 heads of 32.
The attention mask is block-diagonal, so softmax/attention never crosses graphs.

Sharding: data-parallel over graphs — 2 graphs (768 nodes) per core, the four
256x256 projections replicated.  Each core computes, for its 2 graphs:
  Q^T,K^T = (x W + b)^T in [d, n] layout, V in [n, d] layout (+ones column),
  per (graph, head): S^T = K_h^T(stationary) . Q_h^T -> exp -> attn@V via
  V_aug so row 32 of the product is the softmax denominator. The unnormalized
  exp-scores ship to the host (the attention-probabilities output is
  block-diagonal; the host divides by the shipped row sums during assembly).
  The context rows are normalized on-device (per-head reciprocal broadcast via
  a tiny selector matmul) and pushed through W_o (+b_o via ones row).
"""

import os
import sys

for _p in ("/opt/trn_rl_repo", "/root/.axon_site/_ro/trn_rl_repo"):
    if os.path.isdir(_p) and _p not in sys.path:
        sys.path.insert(0, _p)

import numpy as np

import concourse.bass as bass  # noqa: E402
import concourse.tile as tile  # noqa: E402
from concourse import bacc, mybir  # noqa: E402
from concourse.bass_utils import run_bass_kernel_spmd  # noqa: E402
from concourse.masks import make_identity  # noqa: E402

F32 = mybir.dt.float32
F32R = mybir.dt.float32r

N_CORES = 8
B = 16          # graphs
D = 256         # d_model
H = 8           # heads
DK = 32         # head dim
S = 384         # nodes per graph
G = 2           # graphs per core
NL = G * S      # nodes per core = 768
C = S // 128    # 128-chunks per graph = 3
NCH = NL // 128  # node chunks per core = 6
FC = D // 128   # feature chunks = 2
W33 = DK + 1    # head block width in V_aug (ones column appended)
SCALE = 1.0 / float(np.sqrt(DK))

# Flip to False to run all matmuls in full-precision fp32 (4x slower on PE).
USE_F32R = True


MMDT = F32R if USE_F32R else F32


def _build_module():
    nc = bacc.Bacc("TRN2", target_bir_lowering=False, debug=False,
                   num_devices=N_CORES)

    xq_d = nc.dram_tensor("xq", [NL, D], F32, kind="ExternalInput")
    xkv_d = nc.dram_tensor("xkv", [NL, D], F32, kind="ExternalInput")
    w_d = {k: nc.dram_tensor(k, [D, D], F32, kind="ExternalInput")
           for k in ("wq", "wk", "wv", "wo")}
    b_d = {k: nc.dram_tensor(k, [1, D], F32, kind="ExternalInput")
           for k in ("bq", "bk", "bv", "bo")}
    sel_d = nc.dram_tensor("sel4", [4, 128], F32, kind="ExternalInput")

    out_d = nc.dram_tensor("outp", [NL, D], F32, kind="ExternalOutput")
    exps_d = nc.dram_tensor("exps", [G, H, C, 128, S], MMDT, kind="ExternalOutput")
    rsum_d = nc.dram_tensor("rowsum", [H, NL], F32, kind="ExternalOutput")

    with tile.TileContext(nc) as tc:
        with (
            nc.allow_low_precision(reason="fp32r matmul operand rounding"),
            tc.tile_pool(name="sing", bufs=1) as sing,
            tc.tile_pool(name="expp", bufs=4) as expp,
            tc.tile_pool(name="pssc", bufs=2, space="PSUM") as pssc,
            tc.tile_pool(name="pssm", bufs=2, space="PSUM") as pssm,
        ):
            # ---- loads ----
            xq_nm = sing.tile([128, NCH, D], F32)
            nc.sync.dma_start(out=xq_nm, in_=xq_d[:].rearrange("(c p) f -> p c f", p=128))
            xkv_nm = sing.tile([128, NCH, D], F32)
            nc.sync.dma_start(out=xkv_nm, in_=xkv_d[:].rearrange("(c p) f -> p c f", p=128))

            # weights/biases are DMA'd in fp32 then rounded to the matmul
            # dtype with an engine copy (walrus requires fp32r matmul operands
            # to be produced pre-rounded).
            w_sb = {}
            b_sb = {}
            for k in ("wq", "wk", "wv", "wo"):
                w_f = sing.tile([128, FC, D], F32, name=f"wf_{k}")
                nc.sync.dma_start(out=w_f, in_=w_d[k][:].rearrange("(c p) f -> p c f", p=128))
                w_sb[k] = sing.tile([128, FC, D], MMDT, name=f"w_{k}")
                nc.vector.tensor_copy(w_sb[k], w_f)
            for k in ("bq", "bk", "bv", "bo"):
                b_f = sing.tile([1, D], F32, name=f"bf_{k}")
                nc.sync.dma_start(out=b_f, in_=b_d[k][:])
                b_sb[k] = sing.tile([1, D], MMDT, name=f"b_{k}")
                nc.vector.tensor_copy(b_sb[k], b_f)
            sel_f = sing.tile([4, 128], F32)
            nc.sync.dma_start(out=sel_f, in_=sel_d[:])
            sel_sb = sing.tile([4, 128], MMDT)
            nc.vector.tensor_copy(sel_sb, sel_f)

            ones_f = sing.tile([1, NL], F32)
            nc.vector.memset(ones_f, 1.0)
            ones_row = sing.tile([1, NL], MMDT)
            nc.vector.tensor_copy(ones_row, ones_f)
            ones_col_f = sing.tile([128, H], F32)
            nc.vector.memset(ones_col_f, 1.0)
            ident = sing.tile([128, 128], F32)
            make_identity(nc, ident)

            # ---- transpose x to [feature, node] layout ----
            xqT = sing.tile([128, FC, NL], MMDT)
            xkvT = sing.tile([128, FC, NL], MMDT)
            for src, dst in ((xq_nm, xqT), (xkv_nm, xkvT)):
                for fc in range(FC):
                    for c in range(NCH):
                        tp = pssm.tile([128, 128], F32, tag="ps", name="tp")
                        nc.tensor.transpose(tp, src[:, c, fc * 128:(fc + 1) * 128], ident)
                        nc.vector.tensor_copy(dst[:, fc, c * 128:(c + 1) * 128], tp)

            # ---- Q^T / K^T projections ([d, n] layout), bias via ones row ----
            qT = sing.tile([128, FC, NL], MMDT)
            kT = sing.tile([128, FC, NL], MMDT)
            for dst, w, bkey, xT in ((qT, w_sb["wq"], "bq", xqT),
                                     (kT, w_sb["wk"], "bk", xkvT)):
                for fc in range(FC):
                    for nb in range(G):
                        pp = pssm.tile([128, S], F32, tag="ps", name="pp")
                        for kc in range(FC + 1):
                            if kc < FC:
                                lhsT = w[:, kc, fc * 128:(fc + 1) * 128]
                                rhs = xT[:, kc, nb * S:(nb + 1) * S]
                            else:
                                lhsT = b_sb[bkey][:, fc * 128:(fc + 1) * 128]
                                rhs = ones_row[:, nb * S:(nb + 1) * S]
                            nc.tensor.matmul(pp, lhsT, rhs,
                                             start=(kc == 0), stop=(kc == FC))
                        nc.vector.tensor_copy(dst[:, fc, nb * S:(nb + 1) * S], pp)

            # ---- V in [node, feature] layout, packed per head with a ones col ----
            vaug = sing.tile([128, NCH, H * W33], MMDT)
            for c in range(NCH):
                nc.vector.tensor_copy(
                    vaug[:, c, :].rearrange("p (h w) -> p h w", w=W33)[:, :, DK:DK + 1],
                    ones_col_f.unsqueeze(2))
                vp = pssm.tile([128, D], F32, tag="ps", name="vp")
                for kc in range(FC + 1):
                    if kc < FC:
                        lhsT = xkvT[:, kc, c * 128:(c + 1) * 128]
                        rhs = w_sb["wv"][:, kc, :]
                    else:
                        lhsT = ones_row[:, c * 128:(c + 1) * 128]
                        rhs = b_sb["bv"]
                    nc.tensor.matmul(vp, lhsT, rhs,
                                     start=(kc == 0), stop=(kc == FC))
                nc.vector.tensor_copy(
                    vaug[:, c, :].rearrange("p (h w) -> p h w", w=W33)[:, :, 0:DK],
                    vp.rearrange("p (h w) -> p h w", w=DK))

            # ---- per (graph, head) attention ----
            aT = sing.tile([128, FC, NL], F32)      # unnormalized context, [d, n]
            # Softmax denominators come out of the attn@V product on PSUM
            # partition 32; engine APs can only start at partition 0/32/64/96,
            # so stage them all on partition 32 and use SBUF->SBUF DMA (which
            # has no partition-base restriction) to regroup them for the
            # reciprocal-broadcast matmul.
            rsS = sing.tile([DK + 1, H * NL], F32)
            rs4 = [sing.tile([4, NL], F32, name=f"rs4_{i}") for i in range(FC)]
            rr = [sing.tile([4, NL], MMDT, name=f"rr{i}") for i in range(FC)]
            for g in range(G):
                for h in range(H):
                    fc, po = h // 4, (h % 4) * DK
                    sc = pssc.tile([128, C, 512], F32, name="sc")
                    for c in range(C):
                        lhsT = kT[po:po + DK, fc, g * S + c * 128: g * S + (c + 1) * 128]
                        rhs = qT[po:po + DK, fc, g * S:(g + 1) * S]
                        nc.tensor.matmul(sc[:, c, 0:S], lhsT, rhs,
                                         start=True, stop=True,
                                         tile_position=(po, 0))
                    ex = expp.tile([128, C, S], MMDT, name="ex")
                    nc.scalar.activation(ex, sc[:, :, 0:S],
                                         mybir.ActivationFunctionType.Exp, scale=SCALE)
                    nc.sync.dma_start(out=exps_d[g, h].rearrange("c p q -> p c q"), in_=ex)
                    av = pssm.tile([W33, S], F32, tag="ps", name="av")
                    for c in range(C):
                        nc.tensor.matmul(av, vaug[:, g * C + c, h * W33:(h + 1) * W33],
                                         ex[:, c, :], start=(c == 0), stop=(c == C - 1))
                    nc.vector.tensor_copy(aT[po:po + DK, fc, g * S:(g + 1) * S], av[0:DK, :])
                    nc.vector.tensor_copy(
                        rsS[DK:DK + 1, h * NL + g * S: h * NL + (g + 1) * S],
                        av[DK:DK + 1, :])
            nc.sync.dma_start(out=rsum_d[:],
                              in_=rsS[DK:DK + 1, :].rearrange("p (h n) -> p h n", n=NL))
            for fc in range(FC):
                for j in range(4):
                    hh = fc * 4 + j
                    nc.sync.dma_start(out=rs4[fc][j:j + 1, :],
                                      in_=rsS[DK:DK + 1, hh * NL:(hh + 1) * NL])
                nc.vector.reciprocal(rr[fc], rs4[fc])

            # ---- normalize context per head, then output projection ----
            anorm = sing.tile([128, FC, NL], MMDT)
            for fc in range(FC):
                for nb in range(G):
                    rp = pssm.tile([128, S], F32, tag="ps", name="rp")
                    nc.tensor.matmul(rp, sel_sb, rr[fc][:, nb * S:(nb + 1) * S],
                                     start=True, stop=True)
                    nc.vector.tensor_mul(anorm[:, fc, nb * S:(nb + 1) * S],
                                         aT[:, fc, nb * S:(nb + 1) * S], rp)

            out_sb = sing.tile([128, NCH, D], F32)
            for c in range(NCH):
                op = pssm.tile([128, D], F32, tag="ps", name="op")
                for kc in range(FC + 1):
                    if kc < FC:
                        lhsT = anorm[:, kc, c * 128:(c + 1) * 128]
                        rhs = w_sb["wo"][:, kc, :]
                    else:
                        lhsT = ones_row[:, c * 128:(c + 1) * 128]
                        rhs = b_sb["bo"]
                    nc.tensor.matmul(op, lhsT, rhs,
                                     start=(kc == 0), stop=(kc == FC))
                nc.vector.tensor_copy(out_sb[:, c, :], op)
            nc.sync.dma_start(out=out_d[:].rearrange("(c p) f -> p c f", p=128), in_=out_sb)

    nc.compile()
    return nc


_NC = None


def _get_nc():
    global _NC
    if _NC is None:
        _NC = _build_module()
    return _NC


_SEL4 = np.zeros((4, 128), np.float32)
for _j in range(4):
    _SEL4[_j, _j * DK:(_j + 1) * DK] = 1.0


def _numpy_fallback(x_q, x_kv, Wq, bq, Wk, bk, Wv, bv, Wo, bo, batch_q, batch_kv):
    """Plain numpy reference path for input patterns the device kernel doesn't
    cover (non-uniform graph sizes). Mirrors the reference math in fp32."""
    n_q = x_q.shape[0]
    q = (x_q @ Wq + bq).reshape(n_q, H, DK).transpose(1, 0, 2)
    k = (x_kv @ Wk + bk).reshape(-1, H, DK).transpose(1, 0, 2)
    v = (x_kv @ Wv + bv).reshape(-1, H, DK).transpose(1, 0, 2)
    mask = batch_q[:, None] == batch_kv[None, :]
    scores = np.einsum("hqd,hkd->hqk", q, k).astype(np.float32) * np.float32(SCALE)
    scores = np.where(mask[None], scores, np.float32(-1e30))
    scores -= scores.max(axis=-1, keepdims=True)
    e = np.exp(scores, dtype=np.float32)
    attn = e / e.sum(axis=-1, keepdims=True)
    out = np.einsum("hqk,hkd->hqd", attn, v).transpose(1, 0, 2).reshape(n_q, D)
    out = (out @ Wo + bo).astype(np.float32)
    return out, attn[None].astype(np.float32)


def _run_device(in_maps, trace=False, tmpdir=None):
    nc = _get_nc()
    return run_bass_kernel_spmd(nc, in_maps, list(range(N_CORES)),
                                trace=trace, tmpdir=tmpdir)


def kernel(x_q, x_kv, Wq, bq, Wk, bk, Wv, bv, Wo, bo, batch_q, batch_kv,
           _trace=False, _tmpdir=None, _return_raw=False):
    x_q = np.ascontiguousarray(np.asarray(x_q, np.float32))
    x_kv = np.ascontiguousarray(np.asarray(x_kv, np.float32))
    Wq, Wk, Wv, Wo = (np.ascontiguousarray(np.asarray(w, np.float32))
                      for w in (Wq, Wk, Wv, Wo))
    bq, bk, bv, bo = (np.ascontiguousarray(np.asarray(b, np.float32).reshape(1, D))
                      for b in (bq, bk, bv, bo))
    batch_q = np.asarray(batch_q)
    batch_kv = np.asarray(batch_kv)

    pattern = np.repeat(np.arange(B), S)
    if (x_q.shape != (B * S, D) or x_kv.shape != (B * S, D)
            or not np.array_equal(batch_q, pattern)
            or not np.array_equal(batch_kv, pattern)):
        return _numpy_fallback(x_q, x_kv, Wq, bq[0], Wk, bk[0], Wv, bv[0],
                               Wo, bo[0], batch_q, batch_kv)

    in_maps = []
    for i in range(N_CORES):
        rows = slice(i * NL, (i + 1) * NL)
        in_maps.append({
            "xq": x_q[rows], "xkv": x_kv[rows],
            "wq": Wq, "wk": Wk, "wv": Wv, "wo": Wo,
            "bq": bq, "bk": bk, "bv": bv, "bo": bo,
            "sel4": _SEL4,
        })
    res = _run_device(in_maps, trace=_trace, tmpdir=_tmpdir)

    out = np.concatenate([res.results[i]["outp"] for i in range(N_CORES)], axis=0)
    attn = np.zeros((1, H, B * S, B * S), np.float32)
    for i in range(N_CORES):
        exps = res.results[i]["exps"]          # [G, H, C, 128, S]  (k-major, q-minor)
        rsum = res.results[i]["rowsum"]        # [H, NL]
        for g in range(G):
            gg = i * G + g
            blk = exps[g].reshape(H, S, S)     # [h, k, q]
            rs = rsum[:, g * S:(g + 1) * S]    # [h, q]
            sl = slice(gg * S, (gg + 1) * S)
            attn[0, :, sl, sl] = (blk / rs[:, None, :]).transpose(0, 2, 1)
    if _return_raw:
        return (out, attn), res
    return out, attn


# revision 13
# speedup vs baseline: 1.2884x; 1.2884x over previous
"""Block-diagonal (per-graph) multi-head attention for Trainium2, SPMD over 8 cores.

Problem: nn_MultiHeadAttention (sparse_attention). N=6144 nodes in B=16 graphs
of 384 nodes each (batch ids are sorted and uniform), d_model=256, 8 heads of 32.
The attention mask is block-diagonal, so softmax/attention never crosses graphs.

Sharding: data-parallel over graphs — 2 graphs (768 nodes) per core, the four
256x256 projections replicated.  Each core computes, for its 2 graphs:
  Q^T,K^T = (x W + b)^T in [d, n] layout, V in [n, d] layout (+ones column),
  per (graph, head): S^T = K_h^T(stationary) . Q_h^T -> exp -> attn@V via
  V_aug so one extra product row is the softmax denominator. The unnormalized
  exp-scores ship to the host (the attention-probabilities output is
  block-diagonal; the host divides by the shipped row sums during assembly).
  The context rows are normalized on-device (per-head reciprocal broadcast via
  a tiny selector matmul) and pushed through W_o; b_o is added on the host.

Heads are processed in pairs with explicit PE tile positions: the two heads'
score matmuls (K=32) go to different PE row groups and their attn@V matmuls
(M=33) to different column groups, so each pair streams concurrently.
"""

import os
import sys

for _p in ("/opt/trn_rl_repo", "/root/.axon_site/_ro/trn_rl_repo"):
    if os.path.isdir(_p) and _p not in sys.path:
        sys.path.insert(0, _p)

import numpy as np

import concourse.bass as bass  # noqa: E402
import concourse.tile as tile  # noqa: E402
from concourse import bacc, mybir  # noqa: E402
from concourse.bass_utils import run_bass_kernel_spmd  # noqa: E402
from concourse.masks import make_identity  # noqa: E402

F32 = mybir.dt.float32
F32R = mybir.dt.float32r

N_CORES = 8
B = 16          # graphs
D = 256         # d_model
H = 8           # heads
DK = 32         # head dim
S = 384         # nodes per graph
G = 2           # graphs per core
NL = G * S      # nodes per core = 768
C = S // 128    # 128-chunks per graph = 3
NCH = NL // 128  # node chunks per core = 6
FC = D // 128   # feature chunks = 2
W33 = DK + 1    # head block width in V_aug (ones column appended)
SCALE = 1.0 / float(np.sqrt(DK))

# Flip to False to run all matmuls in full-precision fp32 (4x slower on PE).
USE_F32R = True
MMDT = F32R if USE_F32R else F32


def _build_module():
    nc = bacc.Bacc("TRN2", target_bir_lowering=False, debug=False,
                   num_devices=N_CORES)

    xq_d = nc.dram_tensor("xq", [NL, D], F32, kind="ExternalInput")
    xkv_d = nc.dram_tensor("xkv", [NL, D], F32, kind="ExternalInput")
    w_d = {k: nc.dram_tensor(k, [D, D], F32, kind="ExternalInput")
           for k in ("wq", "wk", "wv", "wo")}
    b_d = {k: nc.dram_tensor(k, [1, D], F32, kind="ExternalInput")
           for k in ("bq", "bk", "bv")}
    sel_d = nc.dram_tensor("sel4", [4, 128], F32, kind="ExternalInput")

    out_d = nc.dram_tensor("outp", [NL, D], F32, kind="ExternalOutput")
    exps_d = nc.dram_tensor("exps", [G, H, C, 128, S], MMDT, kind="ExternalOutput")
    rsum_d = nc.dram_tensor("rowsum", [H, NL], F32, kind="ExternalOutput")

    with tile.TileContext(nc) as tc:
        with (
            nc.allow_low_precision(reason="fp32r matmul operand rounding"),
            tc.tile_pool(name="sing", bufs=1) as sing,
            tc.tile_pool(name="expp", bufs=4) as expp,
            tc.tile_pool(name="pssc", bufs=2, space="PSUM") as pssc,
            tc.tile_pool(name="pssm", bufs=2, space="PSUM") as pssm,
        ):
            # ---- loads ----
            xq_nm = sing.tile([128, NCH, D], F32)
            nc.sync.dma_start(out=xq_nm, in_=xq_d[:].rearrange("(c p) f -> p c f", p=128))
            xkv_nm = sing.tile([128, NCH, D], F32)
            nc.sync.dma_start(out=xkv_nm, in_=xkv_d[:].rearrange("(c p) f -> p c f", p=128))

            # weights are DMA'd in fp32 then rounded to the matmul dtype with an
            # engine copy (walrus requires fp32r matmul operands pre-rounded).
            w_sb = {}
            for k in ("wq", "wk", "wv", "wo"):
                w_f = sing.tile([128, FC, D], F32, name=f"wf_{k}")
                nc.sync.dma_start(out=w_f, in_=w_d[k][:].rearrange("(c p) f -> p c f", p=128))
                w_sb[k] = sing.tile([128, FC, D], MMDT, name=f"w_{k}")
                nc.vector.tensor_copy(w_sb[k], w_f)
            # bq/bk as per-partition columns [d%128, d//128] for tensor_scalar adds
            bcol = {}
            for k in ("bq", "bk"):
                bcol[k] = sing.tile([128, FC], F32, name=f"bc_{k}")
                nc.sync.dma_start(out=bcol[k],
                                  in_=b_d[k][:].rearrange("o (c p) -> p (o c)", p=128))
            # bv broadcast across partitions (0-stride partition DMA)
            bv_bc = sing.tile([128, D], F32)
            nc.sync.dma_start(out=bv_bc, in_=b_d["bv"][:].to_broadcast((128, D)))
            sel_f = sing.tile([4, 128], F32)
            nc.sync.dma_start(out=sel_f, in_=sel_d[:])
            sel_sb = sing.tile([4, 128], MMDT)
            nc.vector.tensor_copy(sel_sb, sel_f)

            ones_col_f = sing.tile([128, H], F32)
            nc.vector.memset(ones_col_f, 1.0)
            ident = sing.tile([128, 128], F32)
            make_identity(nc, ident)

            # ---- transpose x to [feature, node] layout ----
            xqT = sing.tile([128, FC, NL], MMDT)
            xkvT = sing.tile([128, FC, NL], MMDT)
            for src, dst in ((xq_nm, xqT), (xkv_nm, xkvT)):
                for fc in range(FC):
                    for c in range(NCH):
                        tp = pssm.tile([128, 128], F32, tag="ps", name="tp")
                        nc.tensor.transpose(tp, src[:, c, fc * 128:(fc + 1) * 128], ident)
                        nc.vector.tensor_copy(dst[:, fc, c * 128:(c + 1) * 128], tp)

            # ---- Q^T / K^T projections ([d, n] layout); bias folded into the
            # PSUM->SBUF eviction as a per-partition scalar add ----
            qT = sing.tile([128, FC, NL], MMDT)
            kT = sing.tile([128, FC, NL], MMDT)
            for dst, w, bkey, xT in ((qT, w_sb["wq"], "bq", xqT),
                                     (kT, w_sb["wk"], "bk", xkvT)):
                for fc in range(FC):
                    for nb in range(G):
                        pp = pssm.tile([128, S], F32, tag="ps", name="pp")
                        for kc in range(FC):
                            nc.tensor.matmul(pp, w[:, kc, fc * 128:(fc + 1) * 128],
                                             xT[:, kc, nb * S:(nb + 1) * S],
                                             start=(kc == 0), stop=(kc == FC - 1))
                        nc.vector.tensor_scalar_add(dst[:, fc, nb * S:(nb + 1) * S],
                                                    pp, bcol[bkey][:, fc:fc + 1])

            # ---- V in [node, feature] layout, packed per head with a ones col;
            # bv added via a partition-broadcast tile during eviction ----
            vaug = sing.tile([128, NCH, H * W33], MMDT)
            for c in range(NCH):
                nc.vector.tensor_copy(
                    vaug[:, c, :].rearrange("p (h w) -> p h w", w=W33)[:, :, DK:DK + 1],
                    ones_col_f.unsqueeze(2))
                vp = pssm.tile([128, D], F32, tag="ps", name="vp")
                for kc in range(FC):
                    nc.tensor.matmul(vp, xkvT[:, kc, c * 128:(c + 1) * 128],
                                     w_sb["wv"][:, kc, :],
                                     start=(kc == 0), stop=(kc == FC - 1))
                nc.vector.tensor_add(
                    vaug[:, c, :].rearrange("p (h w) -> p h w", w=W33)[:, :, 0:DK],
                    vp.rearrange("p (h w) -> p h w", w=DK),
                    bv_bc.rearrange("p (h w) -> p h w", w=DK))

            # ---- per (graph, head-pair) attention ----
            aT = sing.tile([128, FC, NL], F32)      # unnormalized context, [d, n]
            # Softmax denominators land on PSUM partitions 32 / 96; stage them
            # all on partition 32 (legal engine AP bases are 0/32/64/96), then
            # redistribute with SBUF->SBUF DMAs for a lane-parallel reciprocal.
            rsS = sing.tile([DK + 1, H * NL], F32)
            for g in range(G):
                for hp in range(4):
                    h0, h1 = 2 * hp, 2 * hp + 1
                    fc = hp // 2
                    po0, po1 = (h0 % 4) * DK, (h1 % 4) * DK
                    sc0 = pssc.tile([128, C, 512], F32, tag="sc", name="sc0")
                    sc1 = pssc.tile([128, C, 512], F32, tag="sc", name="sc1")
                    for c in range(C):
                        ksl = slice(g * S + c * 128, g * S + (c + 1) * 128)
                        qsl = slice(g * S, (g + 1) * S)
                        nc.tensor.matmul(sc0[:, c, 0:S], kT[po0:po0 + DK, fc, ksl],
                                         qT[po0:po0 + DK, fc, qsl],
                                         start=True, stop=True, tile_position=(po0, 0))
                        nc.tensor.matmul(sc1[:, c, 0:S], kT[po1:po1 + DK, fc, ksl],
                                         qT[po1:po1 + DK, fc, qsl],
                                         start=True, stop=True, tile_position=(po1, 0))
                    ex0 = expp.tile([128, C, S], MMDT, tag="ex", name="ex0")
                    ex1 = expp.tile([128, C, S], MMDT, tag="ex", name="ex1")
                    nc.scalar.activation(ex0, sc0[:, :, 0:S],
                                         mybir.ActivationFunctionType.Exp, scale=SCALE)
                    nc.scalar.activation(ex1, sc1[:, :, 0:S],
                                         mybir.ActivationFunctionType.Exp, scale=SCALE)
                    nc.sync.dma_start(out=exps_d[g, h0].rearrange("c p q -> p c q"), in_=ex0)
                    nc.sync.dma_start(out=exps_d[g, h1].rearrange("c p q -> p c q"), in_=ex1)
                    av0 = pssm.tile([W33, S], F32, tag="ps", name="av0")
                    av1 = pssm.tile([W33, S], F32, tag="ps", name="av1")
                    for c in range(C):
                        nc.tensor.matmul(av0,
                                         vaug[:, g * C + c, h0 * W33:(h0 + 1) * W33],
                                         ex0[:, c, :], start=(c == 0), stop=(c == C - 1))
                        nc.tensor.matmul(av1,
                                         vaug[:, g * C + c, h1 * W33:(h1 + 1) * W33],
                                         ex1[:, c, :], start=(c == 0), stop=(c == C - 1))
                    for h, av in ((h0, av0), (h1, av1)):
                        po = (h % 4) * DK
                        nc.vector.tensor_copy(aT[po:po + DK, fc, g * S:(g + 1) * S],
                                              av[0:DK, :])
                        nc.vector.tensor_copy(
                            rsS[DK:DK + 1, h * NL + g * S: h * NL + (g + 1) * S],
                            av[DK:DK + 1, :])
            nc.sync.dma_start(out=rsum_d[:],
                              in_=rsS[DK:DK + 1, :].rearrange("p (h n) -> p h n", n=NL))

            # ---- reciprocal of the 16 row-sum vectors, lane-parallel ----
            # rsP partitions = (h, g, c), free = node-within-chunk
            rsP = sing.tile([H * G * C, 128], F32)
            nc.sync.dma_start(
                out=rsP,
                in_=rsS[DK:DK + 1, :].rearrange("o (h g c p) -> o h g c p",
                                                h=H, g=G, c=C))
            rrP = sing.tile([H * G * C, 128], MMDT)
            nc.vector.reciprocal(rrP, rsP)
            rr = [sing.tile([4, NL], MMDT, name=f"rr{i}") for i in range(FC)]
            for fc in range(FC):
                for j in range(4):
                    hh = fc * 4 + j
                    nc.sync.dma_start(out=rr[fc][j:j + 1, :],
                                      in_=rrP[hh * G * C:(hh + 1) * G * C, :])

            # ---- normalize context per head, then output projection ----
            anorm = sing.tile([128, FC, NL], MMDT)
            for fc in range(FC):
                for nb in range(G):
                    rp = pssm.tile([128, S], F32, tag="ps", name="rp")
                    nc.tensor.matmul(rp, sel_sb, rr[fc][:, nb * S:(nb + 1) * S],
                                     start=True, stop=True)
                    nc.vector.tensor_mul(anorm[:, fc, nb * S:(nb + 1) * S],
                                         aT[:, fc, nb * S:(nb + 1) * S], rp)

            out_sb = sing.tile([128, NCH, D], F32)
            for c in range(NCH):
                op = pssm.tile([128, D], F32, tag="ps", name="op")
                for kc in range(FC):
                    nc.tensor.matmul(op, anorm[:, kc, c * 128:(c + 1) * 128],
                                     w_sb["wo"][:, kc, :],
                                     start=(kc == 0), stop=(kc == FC - 1))
                nc.vector.tensor_copy(out_sb[:, c, :], op)
            nc.sync.dma_start(out=out_d[:].rearrange("(c p) f -> p c f", p=128), in_=out_sb)

    nc.compile()
    return nc


_NC = None


def _get_nc():
    global _NC
    if _NC is None:
        _NC = _build_module()
    return _NC


_SEL4 = np.zeros((4, 128), np.float32)
for _j in range(4):
    _SEL4[_j, _j * DK:(_j + 1) * DK] = 1.0


def _numpy_fallback(x_q, x_kv, Wq, bq, Wk, bk, Wv, bv, Wo, bo, batch_q, batch_kv):
    """Plain numpy reference path for input patterns the device kernel doesn't
    cover (non-uniform graph sizes). Mirrors the reference math in fp32."""
    n_q = x_q.shape[0]
    q = (x_q @ Wq + bq).reshape(n_q, H, DK).transpose(1, 0, 2)
    k = (x_kv @ Wk + bk).reshape(-1, H, DK).transpose(1, 0, 2)
    v = (x_kv @ Wv + bv).reshape(-1, H, DK).transpose(1, 0, 2)
    mask = batch_q[:, None] == batch_kv[None, :]
    scores = np.einsum("hqd,hkd->hqk", q, k).astype(np.float32) * np.float32(SCALE)
    scores = np.where(mask[None], scores, np.float32(-1e30))
    scores -= scores.max(axis=-1, keepdims=True)
    e = np.exp(scores, dtype=np.float32)
    attn = e / e.sum(axis=-1, keepdims=True)
    out = np.einsum("hqk,hkd->hqd", attn, v).transpose(1, 0, 2).reshape(n_q, D)
    out = (out @ Wo + bo).astype(np.float32)
    return out, attn[None].astype(np.float32)


def _run_device(in_maps, trace=False, tmpdir=None):
    nc = _get_nc()
    return run_bass_kernel_spmd(nc, in_maps, list(range(N_CORES)),
                                trace=trace, tmpdir=tmpdir)


def kernel(x_q, x_kv, Wq, bq, Wk, bk, Wv, bv, Wo, bo, batch_q, batch_kv,
           _trace=False, _tmpdir=None, _return_raw=False):
    x_q = np.ascontiguousarray(np.asarray(x_q, np.float32))
    x_kv = np.ascontiguousarray(np.asarray(x_kv, np.float32))
    Wq, Wk, Wv, Wo = (np.ascontiguousarray(np.asarray(w, np.float32))
                      for w in (Wq, Wk, Wv, Wo))
    bq, bk, bv, bo = (np.ascontiguousarray(np.asarray(b, np.float32).reshape(1, D))
                      for b in (bq, bk, bv, bo))
    batch_q = np.asarray(batch_q)
    batch_kv = np.asarray(batch_kv)

    pattern = np.repeat(np.arange(B), S)
    if (x_q.shape != (B * S, D) or x_kv.shape != (B * S, D)
            or not np.array_equal(batch_q, pattern)
            or not np.array_equal(batch_kv, pattern)):
        return _numpy_fallback(x_q, x_kv, Wq, bq[0], Wk, bk[0], Wv, bv[0],
                               Wo, bo[0], batch_q, batch_kv)

    in_maps = []
    for i in range(N_CORES):
        rows = slice(i * NL, (i + 1) * NL)
        in_maps.append({
            "xq": x_q[rows], "xkv": x_kv[rows],
            "wq": Wq, "wk": Wk, "wv": Wv, "wo": Wo,
            "bq": bq, "bk": bk, "bv": bv,
            "sel4": _SEL4,
        })
    res = _run_device(in_maps, trace=_trace, tmpdir=_tmpdir)

    out = np.concatenate([res.results[i]["outp"] for i in range(N_CORES)], axis=0)
    out += bo  # b_o is a per-feature add after the output projection
    attn = np.zeros((1, H, B * S, B * S), np.float32)
    for i in range(N_CORES):
        exps = res.results[i]["exps"]          # [G, H, C, 128, S]  (k-major, q-minor)
        rsum = res.results[i]["rowsum"]        # [H, NL]
        for g in range(G):
            gg = i * G + g
            blk = exps[g].reshape(H, S, S)     # [h, k, q]
            rs = rsum[:, g * S:(g + 1) * S]    # [h, q]
            sl = slice(gg * S, (gg + 1) * S)
            attn[0, :, sl, sl] = (blk / rs[:, None, :]).transpose(0, 2, 1)
    if _return_raw:
        return (out, attn), res
    return out, attn


# revision 16
# speedup vs baseline: 1.4158x; 1.0989x over previous
"""Block-diagonal (per-graph) multi-head attention for Trainium2, SPMD over 8 cores.

Problem: nn_MultiHeadAttention (sparse_attention). N=6144 nodes in B=16 graphs
of 384 nodes each (batch ids are sorted and uniform), d_model=256, 8 heads of 32.
The attention mask is block-diagonal, so softmax/attention never crosses graphs.

Sharding: data-parallel over graphs — 2 graphs (768 nodes) per core, the four
256x256 projections replicated.  Each core computes, for its 2 graphs:
  Q^T,K^T = (x W + b)^T in [d, n] layout, V in [n, d] layout (+ones column),
  per (graph, head): S^T = K_h^T(stationary) . Q_h^T -> exp -> attn@V via
  V_aug so one extra product row is the softmax denominator. The unnormalized
  exp-scores ship to the host (the attention-probabilities output is
  block-diagonal; the host divides by the shipped row sums during assembly).
  The context rows are normalized on-device (per-head reciprocal broadcast via
  a tiny selector matmul) and pushed through W_o; b_o is added on the host.

Heads are processed in pairs with explicit PE tile positions: the two heads'
score matmuls (K=32) go to different PE row groups and their attn@V matmuls
(M=33) to different column groups, so each pair streams concurrently.
"""

import os
import sys

for _p in ("/opt/trn_rl_repo", "/root/.axon_site/_ro/trn_rl_repo"):
    if os.path.isdir(_p) and _p not in sys.path:
        sys.path.insert(0, _p)

import numpy as np

import concourse.bass as bass  # noqa: E402
import concourse.tile as tile  # noqa: E402
from concourse import bacc, mybir  # noqa: E402
from concourse.bass_utils import run_bass_kernel_spmd  # noqa: E402
from concourse.masks import make_identity  # noqa: E402

F32 = mybir.dt.float32
F32R = mybir.dt.float32r

N_CORES = 8
B = 16          # graphs
D = 256         # d_model
H = 8           # heads
DK = 32         # head dim
S = 384         # nodes per graph
G = 2           # graphs per core
NL = G * S      # nodes per core = 768
C = S // 128    # 128-chunks per graph = 3
NCH = NL // 128  # node chunks per core = 6
FC = D // 128   # feature chunks = 2
W33 = DK + 1    # head block width in V_aug (ones column appended)
SCALE = 1.0 / float(np.sqrt(DK))

# Flip to False to run all matmuls in full-precision fp32 (4x slower on PE).
USE_F32R = True
MMDT = F32R if USE_F32R else F32


def _build_module():
    nc = bacc.Bacc("TRN2", target_bir_lowering=False, debug=False,
                   num_devices=N_CORES)

    xqt_d = nc.dram_tensor("xqt", [D, NL], MMDT, kind="ExternalInput")
    xkvt_d = nc.dram_tensor("xkvt", [D, NL], MMDT, kind="ExternalInput")
    wall_d = nc.dram_tensor("wall", [4 * D, D], F32, kind="ExternalInput")
    ball_d = nc.dram_tensor("ball", [3, D], F32, kind="ExternalInput")
    sel_d = nc.dram_tensor("sel4", [4, 128], F32, kind="ExternalInput")

    out_d = nc.dram_tensor("outp", [NL, D], F32, kind="ExternalOutput")
    exps_d = nc.dram_tensor("exps", [G, H, C, 128, S], MMDT, kind="ExternalOutput")
    rsum_d = nc.dram_tensor("rowsum", [H, NL], F32, kind="ExternalOutput")

    with tile.TileContext(nc) as tc:
        with (
            nc.allow_low_precision(reason="fp32r matmul operand rounding"),
            tc.tile_pool(name="sing", bufs=1) as sing,
            tc.tile_pool(name="expp", bufs=4) as expp,
            tc.tile_pool(name="pssc", bufs=2, space="PSUM") as pssc,
            tc.tile_pool(name="pssm", bufs=2, space="PSUM") as pssm,
        ):
            # ---- loads ----
            # weights arrive as one stacked tensor, rounded to the matmul dtype
            # in a single cast (walrus requires fp32r matmul operands pre-rounded).
            U16 = mybir.dt.uint16
            w_f = sing.tile([128, 4 * FC, D], F32)
            nc.sync.dma_start(out=w_f,
                              in_=wall_d[:].rearrange("(w c p) f -> p (w c) f", p=128, c=FC))
            w_all = sing.tile([128, 4 * FC, D], MMDT)
            nc.vector.tensor_copy(w_all, w_f)
            w_sb = {k: w_all[:, i * FC:(i + 1) * FC, :]
                    for i, k in enumerate(("wq", "wk", "wv", "wo"))}
            # bq/bk as per-partition columns [d%128, (b, d//128)] for scalar adds
            bcol = sing.tile([128, 2, FC], F32)
            nc.sync.dma_start(out=bcol,
                              in_=ball_d[0:2, :].rearrange("b (c p) -> p b c", p=128))
            # bv broadcast across partitions (0-stride partition DMA)
            bv_bc = sing.tile([128, D], F32)
            nc.sync.dma_start(out=bv_bc, in_=ball_d[2:3, :].to_broadcast((128, D)))
            sel_f = sing.tile([4, 128], F32)
            nc.sync.dma_start(out=sel_f, in_=sel_d[:])
            sel_sb = sing.tile([4, 128], MMDT)
            nc.vector.tensor_copy(sel_sb, sel_f)

            ones_col_f = sing.tile([128, H], F32)
            nc.vector.memset(ones_col_f, 1.0)

            # ---- x arrives pre-transposed from the host ([feature, node]) ----
            xqT = sing.tile([128, FC, NL], MMDT)
            xkvT = sing.tile([128, FC, NL], MMDT)
            nc.sync.dma_start(out=xqT, in_=xqt_d[:].rearrange("(c p) n -> p c n", p=128))
            nc.sync.dma_start(out=xkvT, in_=xkvt_d[:].rearrange("(c p) n -> p c n", p=128))

            # ---- Q^T / K^T projections ([d, n] layout); bias folded into the
            # PSUM->SBUF eviction as a per-partition scalar add ----
            qT = sing.tile([128, FC, NL], MMDT)
            kT = sing.tile([128, FC, NL], MMDT)
            for dst, w, bi, xT in ((qT, w_sb["wq"], 0, xqT),
                                   (kT, w_sb["wk"], 1, xkvT)):
                for fc in range(FC):
                    for nb in range(G):
                        pp = pssm.tile([128, S], F32, tag="ps", name="pp")
                        for kc in range(FC):
                            nc.tensor.matmul(pp, w[:, kc, fc * 128:(fc + 1) * 128],
                                             xT[:, kc, nb * S:(nb + 1) * S],
                                             start=(kc == 0), stop=(kc == FC - 1))
                        nc.vector.tensor_scalar_add(dst[:, fc, nb * S:(nb + 1) * S],
                                                    pp, bcol[:, bi, fc:fc + 1])

            # ---- V in [node, feature] layout, packed per head with a ones col;
            # bv added via a partition-broadcast tile during eviction ----
            vaug = sing.tile([128, NCH, H * W33], MMDT)
            for c in range(NCH):
                nc.vector.tensor_copy(
                    vaug[:, c, :].rearrange("p (h w) -> p h w", w=W33)[:, :, DK:DK + 1],
                    ones_col_f.unsqueeze(2))
                vp = pssm.tile([128, D], F32, tag="ps", name="vp")
                for kc in range(FC):
                    nc.tensor.matmul(vp, xkvT[:, kc, c * 128:(c + 1) * 128],
                                     w_sb["wv"][:, kc, :],
                                     start=(kc == 0), stop=(kc == FC - 1))
                nc.vector.tensor_add(
                    vaug[:, c, :].rearrange("p (h w) -> p h w", w=W33)[:, :, 0:DK],
                    vp.rearrange("p (h w) -> p h w", w=DK),
                    bv_bc.rearrange("p (h w) -> p h w", w=DK))

            # ---- per (graph, head-pair) attention ----
            aT = sing.tile([128, FC, NL], F32)      # unnormalized context, [d, n]
            # Softmax denominators land on PSUM partitions 32 / 96; stage them
            # all on partition 32 (legal engine AP bases are 0/32/64/96), then
            # redistribute with SBUF->SBUF DMAs for a lane-parallel reciprocal.
            rsS = sing.tile([DK + 1, H * NL], F32)
            for g in range(G):
                for hp in range(4):
                    h0, h1 = 2 * hp, 2 * hp + 1
                    fc = hp // 2
                    po0, po1 = (h0 % 4) * DK, (h1 % 4) * DK
                    sc0 = pssc.tile([128, C, 512], F32, tag="sc", name="sc0")
                    sc1 = pssc.tile([128, C, 512], F32, tag="sc", name="sc1")
                    for c in range(C):
                        ksl = slice(g * S + c * 128, g * S + (c + 1) * 128)
                        qsl = slice(g * S, (g + 1) * S)
                        nc.tensor.matmul(sc0[:, c, 0:S], kT[po0:po0 + DK, fc, ksl],
                                         qT[po0:po0 + DK, fc, qsl],
                                         start=True, stop=True, tile_position=(po0, 0))
                        nc.tensor.matmul(sc1[:, c, 0:S], kT[po1:po1 + DK, fc, ksl],
                                         qT[po1:po1 + DK, fc, qsl],
                                         start=True, stop=True, tile_position=(po1, 0))
                    exp2 = expp.tile([128, 2, C, S], MMDT, tag="ex", name="exp2")
                    ex0, ex1 = exp2[:, 0], exp2[:, 1]
                    nc.scalar.activation(ex0, sc0[:, :, 0:S],
                                         mybir.ActivationFunctionType.Exp, scale=SCALE)
                    nc.scalar.activation(ex1, sc1[:, :, 0:S],
                                         mybir.ActivationFunctionType.Exp, scale=SCALE)
                    nc.gpsimd.dma_start(
                        out=exps_d[g, h0:h0 + 2].rearrange("h c p q -> p h c q"),
                        in_=exp2)
                    av0 = pssm.tile([W33, S], F32, tag="ps", name="av0")
                    av1 = pssm.tile([W33, S], F32, tag="ps", name="av1")
                    for c in range(C):
                        nc.tensor.matmul(av0,
                                         vaug[:, g * C + c, h0 * W33:(h0 + 1) * W33],
                                         ex0[:, c, :], start=(c == 0), stop=(c == C - 1))
                        nc.tensor.matmul(av1,
                                         vaug[:, g * C + c, h1 * W33:(h1 + 1) * W33],
                                         ex1[:, c, :], start=(c == 0), stop=(c == C - 1))
                    for h, av in ((h0, av0), (h1, av1)):
                        po = (h % 4) * DK
                        nc.vector.tensor_copy(aT[po:po + DK, fc, g * S:(g + 1) * S],
                                              av[0:DK, :])
                        nc.vector.tensor_copy(
                            rsS[DK:DK + 1, h * NL + g * S: h * NL + (g + 1) * S],
                            av[DK:DK + 1, :])
            nc.scalar.dma_start(out=rsum_d[:],
                                in_=rsS[DK:DK + 1, :].rearrange("p (h n) -> p h n", n=NL))

            # ---- reciprocal of the 16 row-sum vectors, lane-parallel ----
            # rsP partitions = (h, g, c), free = node-within-chunk
            rsP = sing.tile([H * G * C, 128], F32)
            nc.scalar.dma_start(
                out=rsP,
                in_=rsS[DK:DK + 1, :].rearrange("o (h g c p) -> o h g c p",
                                                h=H, g=G, c=C))
            rrP = sing.tile([H * G * C, 128], MMDT)
            nc.vector.reciprocal(rrP, rsP)
            rr = [sing.tile([4, NL], MMDT, name=f"rr{i}") for i in range(FC)]
            for fc in range(FC):
                for j in range(4):
                    hh = fc * 4 + j
                    nc.scalar.dma_start(out=rr[fc][j:j + 1, :],
                                        in_=rrP[hh * G * C:(hh + 1) * G * C, :])

            # ---- normalize context per head, then output projection ----
            anorm = sing.tile([128, FC, NL], MMDT)
            for fc in range(FC):
                for nb in range(G):
                    rp = pssm.tile([128, S], F32, tag="ps", name="rp")
                    nc.tensor.matmul(rp, sel_sb, rr[fc][:, nb * S:(nb + 1) * S],
                                     start=True, stop=True)
                    nc.vector.tensor_mul(anorm[:, fc, nb * S:(nb + 1) * S],
                                         aT[:, fc, nb * S:(nb + 1) * S], rp)

            out_sb = sing.tile([128, NCH, D], F32)
            for c in range(NCH):
                op = pssm.tile([128, D], F32, tag="ps", name="op")
                for kc in range(FC):
                    nc.tensor.matmul(op, anorm[:, kc, c * 128:(c + 1) * 128],
                                     w_sb["wo"][:, kc, :],
                                     start=(kc == 0), stop=(kc == FC - 1))
                nc.vector.tensor_copy(out_sb[:, c, :], op)
            nc.scalar.dma_start(out=out_d[:].rearrange("(c p) f -> p c f", p=128), in_=out_sb)

    nc.compile()
    return nc


_NC = None


def _get_nc():
    global _NC
    if _NC is None:
        _NC = _build_module()
    return _NC


_SEL4 = np.zeros((4, 128), np.float32)
for _j in range(4):
    _SEL4[_j, _j * DK:(_j + 1) * DK] = 1.0


def _numpy_fallback(x_q, x_kv, Wq, bq, Wk, bk, Wv, bv, Wo, bo, batch_q, batch_kv):
    """Plain numpy reference path for input patterns the device kernel doesn't
    cover (non-uniform graph sizes). Mirrors the reference math in fp32."""
    n_q = x_q.shape[0]
    q = (x_q @ Wq + bq).reshape(n_q, H, DK).transpose(1, 0, 2)
    k = (x_kv @ Wk + bk).reshape(-1, H, DK).transpose(1, 0, 2)
    v = (x_kv @ Wv + bv).reshape(-1, H, DK).transpose(1, 0, 2)
    mask = batch_q[:, None] == batch_kv[None, :]
    scores = np.einsum("hqd,hkd->hqk", q, k).astype(np.float32) * np.float32(SCALE)
    scores = np.where(mask[None], scores, np.float32(-1e30))
    scores -= scores.max(axis=-1, keepdims=True)
    e = np.exp(scores, dtype=np.float32)
    attn = e / e.sum(axis=-1, keepdims=True)
    out = np.einsum("hqk,hkd->hqd", attn, v).transpose(1, 0, 2).reshape(n_q, D)
    out = (out @ Wo + bo).astype(np.float32)
    return out, attn[None].astype(np.float32)


def _run_device(in_maps, trace=False, tmpdir=None):
    nc = _get_nc()
    return run_bass_kernel_spmd(nc, in_maps, list(range(N_CORES)),
                                trace=trace, tmpdir=tmpdir)


def kernel(x_q, x_kv, Wq, bq, Wk, bk, Wv, bv, Wo, bo, batch_q, batch_kv,
           _trace=False, _tmpdir=None, _return_raw=False):
    x_q = np.ascontiguousarray(np.asarray(x_q, np.float32))
    x_kv = np.ascontiguousarray(np.asarray(x_kv, np.float32))
    Wq, Wk, Wv, Wo = (np.ascontiguousarray(np.asarray(w, np.float32))
                      for w in (Wq, Wk, Wv, Wo))
    bq, bk, bv, bo = (np.ascontiguousarray(np.asarray(b, np.float32).reshape(1, D))
                      for b in (bq, bk, bv, bo))
    batch_q = np.asarray(batch_q)
    batch_kv = np.asarray(batch_kv)

    pattern = np.repeat(np.arange(B), S)
    if (x_q.shape != (B * S, D) or x_kv.shape != (B * S, D)
            or not np.array_equal(batch_q, pattern)
            or not np.array_equal(batch_kv, pattern)):
        return _numpy_fallback(x_q, x_kv, Wq, bq[0], Wk, bk[0], Wv, bv[0],
                               Wo, bo[0], batch_q, batch_kv)

    wall = np.ascontiguousarray(np.concatenate([Wq, Wk, Wv, Wo], axis=0))
    ball = np.ascontiguousarray(np.concatenate([bq, bk, bv], axis=0))
    in_maps = []
    for i in range(N_CORES):
        rows = slice(i * NL, (i + 1) * NL)
        in_maps.append({
            "xqt": np.ascontiguousarray(x_q[rows].T),
            "xkvt": np.ascontiguousarray(x_kv[rows].T),
            "wall": wall, "ball": ball,
            "sel4": _SEL4,
        })
    res = _run_device(in_maps, trace=_trace, tmpdir=_tmpdir)

    out = np.concatenate([res.results[i]["outp"] for i in range(N_CORES)], axis=0)
    out += bo  # b_o is a per-feature add after the output projection
    attn = np.zeros((1, H, B * S, B * S), np.float32)
    for i in range(N_CORES):
        exps = res.results[i]["exps"]          # [G, H, C, 128, S]  (k-major, q-minor)
        rsum = res.results[i]["rowsum"]        # [H, NL]
        for g in range(G):
            gg = i * G + g
            blk = exps[g].reshape(H, S, S)     # [h, k, q]
            rs = rsum[:, g * S:(g + 1) * S]    # [h, q]
            sl = slice(gg * S, (gg + 1) * S)
            attn[0, :, sl, sl] = (blk / rs[:, None, :]).transpose(0, 2, 1)
    if _return_raw:
        return (out, attn), res
    return out, attn
